# revision 23
# baseline (speedup 1.0000x reference)
"""GAT layer kernel for Trainium2, 8 NeuronCores — v7.

v6 -> v7 (LDW queue ~97% saturated, but ~90us of per-slot phase-B gaps
  where PE waits on the DVE is_eq+tgs chain):
  - scatter one-hot `ohpl` now comes from the HOST (fp8 DMA, +10.7MB/core)
    instead of a 4.7us/slot DVE is_equal;
  - phase B is interleaved per chunk-group (A-group -> finish -> B-slots)
    so B's DVE/PE work overlaps the next group's LDW stream.
"""

_OLD_DOC = """GAT layer kernel for Trainium2, 8 NeuronCores — v6.

v3 -> v4: killed the device dma_gather (21 x ~34us GpSimd descriptor-gen
  serial stream) by host-gathering h[src] into per-edge streams; per-edge
  eh via LN-stats on the gathered rows; et shrunk to own dst nodes.
  1122us -> 495us.

v4 -> v5: fp8 stationaries — MEASURED NO EFFECT: LDWEIGHTS is ~104ns per
  128x128 stationary regardless of dtype (row-streaming at ~1.2GHz).
  GpSimd tgs-multiply regressed (519us).  Kept: fp8 squares/one-hots
  (halve their DMA), both squares on ACT.

v5 -> v6 (LDWEIGHTS-count is the wall: 6 streams x chunk-count x 104ns):
  - Rebalanced dst-block assignment: 157 blocks packed into 8 cores x 20
    slots (was 21) with per-slot chunk counts TB[s] = ceil(max block size
    in slot / 128); flat-packed streams.  Sum(TB) 651 vs 714 (-8.8%).
  - hg_pk rows are 129-wide with a host 1.0 column: one broadcast multiply
    writes all 129 tgs columns (kills a 1.9us strided CAST per block).
  - fs scale via broadcast tensor_tensor (AP-scalar tensor_scalar was
    ~1.1us); LN constants c_h/c_t/c_r as float immediates.
  - Stat-major flat arrays (suR/s1R/suH/s1H/etd/s2R/s2H) so batched
    finishes read contiguous ranges; ACT calls grouped by function.
"""

import os
import sys

sys.path.insert(0, "/opt/trn_rl_repo")

import numpy as np
import ml_dtypes

import concourse.bacc as bacc
import concourse.bass as bass
import concourse.mybir as mybir
import concourse.tile as tile
from concourse.bass_interp import get_hw_module

F32 = mybir.dt.float32
F16 = mybir.dt.float16
F8 = mybir.dt.float8e4
AF = mybir.ActivationFunctionType
OP = mybir.AluOpType
E4 = ml_dtypes.float8_e4m3

N = 20000
E = 640000
D = 128
NCORES = 8
EPS = 1e-6
NSLOT = 20


# ----------------------------------------------------------------- host prep
def _host_prep(h, r, src, dst, hn_a, hn_b, tn_a, tn_b, rn_a, rn_b,
               head_w, tail_w, rel_w, fc_w, fc_b):
    h = np.asarray(h, np.float32); r = np.asarray(r, np.float32)
    src = np.asarray(src, np.int32); dst = np.asarray(dst, np.int32)

    u_h = np.asarray(hn_a, np.float32) * np.asarray(head_w, np.float32)
    u_t = np.asarray(tn_a, np.float32) * np.asarray(tail_w, np.float32)
    u_r = np.asarray(rn_a, np.float32) * np.asarray(rel_w, np.float32)
    w_h = u_h - u_h.sum() / D
    w_t = u_t - u_t.sum() / D
    w_r = u_r - u_r.sum() / D
    c_h = float((np.asarray(hn_b, np.float32) * head_w).sum())
    c_t = float((np.asarray(tn_b, np.float32) * tail_w).sum())
    c_r = float((np.asarray(rn_b, np.float32) * rel_w).sum())

    perm = np.argsort(dst, kind="stable")
    dst_s = dst[perm]; src_s = src[perm]
    counts = np.bincount(dst, minlength=N)
    cum = np.concatenate([[0], np.cumsum(counts)])

    # --- balanced block -> (core, slot) assignment -----------------------
    nblk = (N + 127) // 128                     # 157
    bcnt = np.array([int(cum[min(b0 + 128, N)] - cum[b0])
                     for b0 in range(0, N, 128)])
    order = np.argsort(-bcnt, kind="stable")    # blocks desc by edge count
    # slot s gets ranks [8s, 8s+8); within a slot, largest block goes to the
    # currently least-loaded core
    assign = -np.ones((NCORES, NSLOT), np.int64)
    load = np.zeros(NCORES, np.int64)
    TB = []
    for s in range(NSLOT):
        grp = list(order[8 * s: 8 * s + 8])
        mx = max((bcnt[bi] for bi in grp), default=0)
        TB.append(max(1, (int(mx) + 127) // 128))
        cores = np.argsort(load, kind="stable")
        for i, bi in enumerate(grp):
            assign[cores[i], s] = bi
            load[cores[i]] += bcnt[bi]
    TB = tuple(TB)
    toff = np.concatenate([[0], np.cumsum(TB)])
    EPKf = int(toff[-1])                        # total chunk slots per core
    ef = 128 * EPKf                             # total edge slots per core

    # zero-padded f16 copies for host-side gathers
    h16z = np.zeros((N + 1, D + 1), np.float16)
    h16z[:N, :D] = h.astype(np.float16)
    h16z[:N, D] = 1.0                           # tgs esum column
    h16zT = np.ascontiguousarray(h16z[:, :D].T)     # [128, N+1]
    r16z = np.zeros((E + 1, D), np.float16)
    r16z[:E] = r.astype(np.float16)

    iota16 = np.broadcast_to(np.arange(128, dtype=np.float16), (128, 128)).copy()
    ident = np.eye(128, dtype=np.float16)

    def wcol(w):
        a = np.zeros((128, 2), np.float16)
        a[:, 0] = w.astype(np.float16); a[:, 1] = 1.0
        return a
    wr16 = wcol(w_r); wh16 = wcol(w_h); wt16 = wcol(w_t)
    ones16 = np.ones((128, 1), np.float16)
    ones2 = np.zeros((128, 2, 2), np.float32)
    ones2[:, 0, 0] = 1.0; ones2[:, 1, 1] = 1.0
    ones2 = ones2.reshape(128, 4).astype(E4)
    fcw16 = np.asarray(fc_w, np.float32).astype(np.float16)
    fcb = np.broadcast_to(np.asarray(fc_b, np.float32), (128, 128)).copy()

    rep = {"iota16": iota16, "ident": ident, "wr16": wr16, "wh16": wh16,
           "wt16": wt16, "ones16": ones16, "ones2": ones2, "fcw16": fcw16,
           "fcb": fcb}

    vidx = np.arange(128, dtype=np.float32)

    in_maps = []
    for k in range(NCORES):
        src_arr = np.full((EPKf, 128), N, np.int64)     # pad -> zero row
        rcol = np.full((EPKf, 128), E, np.int64)
        dstl = np.full((EPKf, 128), -1.0, np.float32)   # [chunk, p]
        hTo = np.zeros((D, NSLOT * 128), np.float16)
        for s in range(NSLOT):
            bi = assign[k, s]
            if bi < 0:
                continue
            b0 = 128 * bi
            e0, e1 = int(cum[b0]), int(cum[min(b0 + 128, N)])
            cnt = e1 - e0
            o = toff[s]
            # edge i (0..cnt) at chunk o + i//128, lane i%128
            fl = np.full(TB[s] * 128, N, np.int64)
            fl[:cnt] = src_s[e0:e1]
            src_arr[o:o + TB[s]] = fl.reshape(TB[s], 128)
            fl = np.full(TB[s] * 128, E, np.int64)
            fl[:cnt] = perm[e0:e1]
            rcol[o:o + TB[s]] = fl.reshape(TB[s], 128)
            fl = np.full(TB[s] * 128, -1.0, np.float32)
            fl[:cnt] = (dst_s[e0:e1] - b0).astype(np.float32)
            dstl[o:o + TB[s]] = fl.reshape(TB[s], 128)
            nn = min(128, N - b0)
            hTo[:, 128 * s:128 * s + nn] = h16zT[:, b0:b0 + nn]

        rT16 = np.ascontiguousarray(r16z[rcol.reshape(-1)].T)
        hgT16 = np.ascontiguousarray(h16zT[:, src_arr.reshape(-1)])
        # hg_pk[p, c, :] = h16z[src of edge (chunk c, lane p)] with ones col
        hg = h16z[src_arr]                               # [c, p, 129]
        hg_pk = np.ascontiguousarray(
            hg.transpose(1, 0, 2).reshape(128, EPKf * (D + 1)))
        dstl_pk = np.ascontiguousarray(dstl.T)           # [128, EPKf]
        oht = (dstl.reshape(1, -1) == vidx[:, None])     # [j, c*128]
        oht = np.ascontiguousarray(oht).astype(E4)
        # scatter one-hot, pk layout: ohpl[p, c, j] = (dstl[c, p] == j)
        ohpl = (dstl[:, :, None] == vidx[None, None, :])  # [c, p, j]
        ohpl = np.ascontiguousarray(
            ohpl.transpose(1, 0, 2).reshape(128, EPKf * 128)).astype(E4)

        in_maps.append(dict(rep, rT16=rT16, hgT16=hgT16, hg_pk=hg_pk,
                            dstl=dstl_pk, oht=oht, ohpl=ohpl, hTo=hTo))
    spec = (TB, round(c_h, 8), round(c_t, 8), round(c_r, 8))
    meta = dict(t_b=spec, assign=assign, ef=ef)
    return in_maps, meta


# ------------------------------------------------------------ device program
def build_program(spec, loop_k=1, for_hw=True):
    TB, c_h, c_t, c_r = spec
    TBm = max(TB)
    toff = [0]
    for t in TB:
        toff.append(toff[-1] + t)
    EPKf = toff[-1]
    ef = 128 * EPKf
    nc = bacc.Bacc("TRN2", target_bir_lowering=False, debug=False,
                   enable_asserts=False, num_devices=NCORES if for_hw else 1)

    dt_rT = nc.dram_tensor("rT16", [128, ef], F16, kind="ExternalInput")
    dt_hgT = nc.dram_tensor("hgT16", [128, ef], F16, kind="ExternalInput")
    dt_hg = nc.dram_tensor("hg_pk", [128, EPKf * (D + 1)], F16, kind="ExternalInput")
    dt_oht = nc.dram_tensor("oht", [128, ef], F8, kind="ExternalInput")
    dt_ohpl = nc.dram_tensor("ohpl", [128, EPKf * 128], F8, kind="ExternalInput")
    dt_hTo = nc.dram_tensor("hTo", [128, NSLOT * 128], F16, kind="ExternalInput")
    dt_ident = nc.dram_tensor("ident", [128, 128], F16, kind="ExternalInput")
    dt_wr = nc.dram_tensor("wr16", [128, 2], F16, kind="ExternalInput")
    dt_wh = nc.dram_tensor("wh16", [128, 2], F16, kind="ExternalInput")
    dt_wt = nc.dram_tensor("wt16", [128, 2], F16, kind="ExternalInput")
    dt_ones = nc.dram_tensor("ones16", [128, 1], F16, kind="ExternalInput")
    dt_ones2 = nc.dram_tensor("ones2", [128, 4], F8, kind="ExternalInput")
    dt_fcw = nc.dram_tensor("fcw16", [128, 128], F16, kind="ExternalInput")
    dt_fcb = nc.dram_tensor("fcb", [128, 128], F32, kind="ExternalInput")
    dt_out = nc.dram_tensor("out", [NSLOT * 128, 128], F32, kind="ExternalOutput")

    with tile.TileContext(nc) as tc:
        with tc.tile_pool(name="const", bufs=1) as cpool:
            ident_sb = cpool.tile([128, 128], F16)
            nc.gpsimd.dma_start(out=ident_sb[:], in_=dt_ident.ap())
            wr_sb = cpool.tile([128, 2], F16)
            nc.gpsimd.dma_start(out=wr_sb[:], in_=dt_wr.ap())
            wh_sb = cpool.tile([128, 2], F16)
            nc.gpsimd.dma_start(out=wh_sb[:], in_=dt_wh.ap())
            wt_sb = cpool.tile([128, 2], F16)
            nc.gpsimd.dma_start(out=wt_sb[:], in_=dt_wt.ap())
            ones_sb = cpool.tile([128, 1], F16)
            nc.gpsimd.dma_start(out=ones_sb[:], in_=dt_ones.ap())
            ones2_sb = cpool.tile([128, 2, 2], F8)
            nc.gpsimd.dma_start(out=ones2_sb[:],
                              in_=dt_ones2.ap().rearrange("p (a b) -> p a b", a=2))
            fcw_sb = cpool.tile([128, 128], F16)
            nc.gpsimd.dma_start(out=fcw_sb[:], in_=dt_fcw.ap())
            fcb_sb = cpool.tile([128, 128], F32)
            nc.gpsimd.dma_start(out=fcb_sb[:], in_=dt_fcb.ap())

            def loop_body():
                with tc.tile_pool(name="stage", bufs=1) as st:
                    et16 = st.tile([128, NSLOT], F16)
                    suR = st.tile([128, EPKf], F32)
                    s1R = st.tile([128, EPKf], F32)
                    suH = st.tile([128, EPKf], F32)
                    s1H = st.tile([128, EPKf], F32)
                    etdA = st.tile([128, EPKf], F32)
                    s2R = st.tile([128, EPKf], F32)
                    s2H = st.tile([128, EPKf], F32)
                    exA = st.tile([128, EPKf], F32)
                    obA = st.tile([128, NSLOT, 128], F16)

                    # ---------- node et stats (own dst blocks only) ----------
                    with tc.tile_pool(name="nwork", bufs=1) as nw, \
                         tc.tile_pool(name="npsum", bufs=1, space="PSUM") as npp:
                        hTo_sb = nw.tile([128, NSLOT * 128], F16)
                        nc.sync.dma_start(out=hTo_sb[:], in_=dt_hTo.ap())
                        hTo2 = nw.tile([128, NSLOT * 128], F8)
                        nc.vector.tensor_mul(out=hTo2[:], in0=hTo_sb[:], in1=hTo_sb[:])
                        psN = npp.tile([128, 3 * NSLOT], F32)
                        for c in range(NSLOT):
                            nc.tensor.matmul(psN[:, 2 * c:2 * c + 2],
                                             hTo_sb[:, 128 * c:128 * (c + 1)],
                                             wt_sb[:], start=True, stop=True)
                            nc.tensor.matmul(psN[:, 2 * NSLOT + c:2 * NSLOT + c + 1],
                                             hTo2[:, 128 * c:128 * (c + 1)],
                                             ones_sb[:], start=True, stop=True)
                        sN = nw.tile([128, 3 * NSLOT], F32)
                        nc.vector.tensor_copy(out=sN[:], in_=psN[:])
                        suN = sN[:, 0:2 * NSLOT:2]
                        s1N = sN[:, 1:2 * NSLOT:2]
                        s2N = sN[:, 2 * NSLOT:3 * NSLOT]
                        t0 = nw.tile([128, NSLOT], F32)
                        nc.vector.tensor_mul(out=t0[:], in0=s1N, in1=s1N)
                        nc.vector.tensor_scalar_mul(out=t0[:], in0=t0[:], scalar1=-1.0 / 128.0)
                        nc.vector.tensor_tensor(out=t0[:], in0=t0[:], in1=s2N, op=OP.add)
                        nc.scalar.activation(out=t0[:], in_=t0[:], func=AF.Sqrt, scale=1.0 / 127.0)
                        nc.vector.tensor_scalar_add(out=t0[:], in0=t0[:], scalar1=EPS)
                        nc.vector.reciprocal(out=t0[:], in_=t0[:])
                        m1 = nw.tile([128, NSLOT], F32)
                        nc.vector.tensor_tensor(out=m1[:], in0=suN, in1=t0[:], op=OP.mult)
                        nc.vector.tensor_scalar_add(out=m1[:], in0=m1[:], scalar1=c_t)
                        nc.scalar.activation(out=et16[:], in_=m1[:], func=AF.Tanh)

                    # ---- phase A: per-edge stats + batched finish -> exA ----
                    CH = [(0, 6), (6, 12), (12, 17), (17, NSLOT)]
                    with tc.tile_pool(name="awork", bufs=3) as aw, \
                         tc.tile_pool(name="bwork", bufs=2) as bw, \
                         tc.tile_pool(name="hgp", bufs=3) as hgp, \
                         tc.tile_pool(name="ohplp", bufs=3) as ohp, \
                         tc.tile_pool(name="bsc", bufs=3) as bsc, \
                         tc.tile_pool(name="efin", bufs=2) as ef_, \
                         tc.tile_pool(name="apsum", bufs=2, space="PSUM") as ap_, \
                         tc.tile_pool(name="bps1", bufs=2, space="PSUM") as bp1, \
                         tc.tile_pool(name="bps2", bufs=2, space="PSUM") as bp2:
                        def phase_a_group(lo, hi):
                            for s in range(lo, hi):
                                tb = TB[s]
                                off = toff[s]
                                rTb = aw.tile([128, TBm * 128], F16, tag="rTb")
                                nc.scalar.dma_start(out=rTb[:, :128 * tb],
                                                    in_=dt_rT.ap()[:, 128 * off:128 * (off + tb)])
                                hgTb = aw.tile([128, TBm * 128], F16, tag="hgTb")
                                nc.sync.dma_start(out=hgTb[:, :128 * tb],
                                                  in_=dt_hgT.ap()[:, 128 * off:128 * (off + tb)])
                                ohtb = aw.tile([128, TBm * 128], F8, tag="ohtb")
                                nc.scalar.dma_start(out=ohtb[:, :128 * tb],
                                                    in_=dt_oht.ap()[:, 128 * off:128 * (off + tb)])
                                psE = ap_.tile([128, 7 * TBm], F32, tag="psE")
                                for t in range(tb):
                                    nc.tensor.matmul(psE[:, 2 * t:2 * t + 2],
                                                     rTb[:, 128 * t:128 * (t + 1)],
                                                     wr_sb[:], start=True, stop=True)
                                    nc.tensor.matmul(psE[:, 2 * tb + 2 * t:2 * tb + 2 * t + 2],
                                                     hgTb[:, 128 * t:128 * (t + 1)],
                                                     wh_sb[:], start=True, stop=True)
                                    nc.tensor.matmul(psE[:, 4 * tb + t:4 * tb + t + 1],
                                                     ohtb[:, 128 * t:128 * (t + 1)],
                                                     et16[:, s:s + 1], start=True, stop=True)
                                sq2 = aw.tile([128, TBm, 2, 128], F8, tag="sq2")
                                nc.scalar.activation(
                                    out=sq2[:, :tb, 0, :],
                                    in_=rTb[:, :128 * tb].rearrange("p (t d) -> p t d", t=tb),
                                    func=AF.Square)
                                nc.scalar.activation(
                                    out=sq2[:, :tb, 1, :],
                                    in_=hgTb[:, :128 * tb].rearrange("p (t d) -> p t d", t=tb),
                                    func=AF.Square)
                                for t in range(tb):
                                    nc.tensor.matmul(psE[:, 5 * tb + 2 * t:5 * tb + 2 * t + 2],
                                                     sq2[:, t, :, :], ones2_sb[:],
                                                     perf_mode=mybir.MatmulPerfMode.DoubleRow,
                                                     start=True, stop=True)
                                for arr, sl in ((suR, slice(0, 2 * tb, 2)),
                                                (s1R, slice(1, 2 * tb, 2)),
                                                (suH, slice(2 * tb, 4 * tb, 2)),
                                                (s1H, slice(2 * tb + 1, 4 * tb, 2)),
                                                (etdA, slice(4 * tb, 5 * tb)),
                                                (s2R, slice(5 * tb, 7 * tb, 2)),
                                                (s2H, slice(5 * tb + 1, 7 * tb, 2))):
                                    nc.vector.tensor_copy(out=arr[:, off:off + tb], in_=psE[:, sl])
                            # batched finish for slots [lo, hi)
                            o0, o1 = toff[lo], toff[hi]
                            cw = o1 - o0
                            tR = ef_.tile([128, 7 * TBm], F32, tag="tR")
                            tRv = tR[:, 0:cw]
                            tH = ef_.tile([128, 7 * TBm], F32, tag="tH")
                            tHv = tH[:, 0:cw]
                            nc.vector.tensor_tensor(out=tRv, in0=s1R[:, o0:o1], in1=s1R[:, o0:o1], op=OP.mult)
                            nc.vector.tensor_scalar_mul(out=tRv, in0=tRv, scalar1=-1.0 / 128.0)
                            nc.vector.tensor_tensor(out=tRv, in0=tRv, in1=s2R[:, o0:o1], op=OP.add)
                            nc.vector.tensor_tensor(out=tHv, in0=s1H[:, o0:o1], in1=s1H[:, o0:o1], op=OP.mult)
                            nc.vector.tensor_scalar_mul(out=tHv, in0=tHv, scalar1=-1.0 / 128.0)
                            nc.vector.tensor_tensor(out=tHv, in0=tHv, in1=s2H[:, o0:o1], op=OP.add)
                            nc.scalar.activation(out=tRv, in_=tRv, func=AF.Sqrt, scale=1.0 / 127.0)
                            nc.scalar.activation(out=tHv, in_=tHv, func=AF.Sqrt, scale=1.0 / 127.0)
                            nc.vector.tensor_scalar_add(out=tRv, in0=tRv, scalar1=EPS)
                            nc.vector.tensor_scalar_add(out=tHv, in0=tHv, scalar1=EPS)
                            nc.vector.reciprocal(out=tRv, in_=tRv)
                            nc.vector.reciprocal(out=tHv, in_=tHv)
                            nc.vector.tensor_tensor(out=tRv, in0=suR[:, o0:o1], in1=tRv, op=OP.mult)
                            nc.vector.tensor_tensor(out=tHv, in0=suH[:, o0:o1], in1=tHv, op=OP.mult)
                            nc.vector.tensor_scalar_add(out=tRv, in0=tRv, scalar1=c_r)
                            nc.vector.tensor_scalar_add(out=tHv, in0=tHv, scalar1=c_h)
                            nc.scalar.activation(out=tRv, in_=tRv, func=AF.Tanh)
                            nc.scalar.activation(out=tHv, in_=tHv, func=AF.Tanh)
                            nc.vector.tensor_tensor(out=tRv, in0=tRv, in1=tHv, op=OP.add)
                            exv = exA[:, o0:o1]
                            nc.vector.tensor_tensor(out=exv, in0=tRv, in1=etdA[:, o0:o1], op=OP.add)
                            nc.scalar.activation(out=exv, in_=exv, func=AF.Exp)
                            nc.vector.tensor_scalar(out=exv, in0=exv, scalar1=1.0,
                                                    scalar2=None, op0=OP.max)

                        def phase_b(s):
                            tb = TB[s]
                            off = toff[s]
                            ohpl = ohp.tile([128, TBm, 128], F8, tag="ohpl")
                            nc.sync.dma_start(
                                out=ohpl[:, :tb, :],
                                in_=dt_ohpl.ap()[:, off * 128:(off + tb) * 128]
                                    .rearrange("p (t d) -> p t d", t=tb))
                            hgb = hgp.tile([128, TBm, D + 1], F16, tag="hgb")
                            nc.sync.dma_start(
                                out=hgb[:, :tb, :],
                                in_=dt_hg.ap()[:, off * (D + 1):(off + tb) * (D + 1)]
                                    .rearrange("p (t d) -> p t d", t=tb))
                            tgs = bw.tile([128, TBm, D + 1], F16, tag="tgs")
                            nc.vector.tensor_tensor(
                                out=tgs[:, :tb, :], in0=hgb[:, :tb, :],
                                in1=exA[:, off:off + tb, None].to_broadcast([128, tb, D + 1]),
                                op=OP.mult)
                            psF = bp1.tile([128, 129], F32, tag="psF")
                            for t in range(tb):
                                nc.tensor.matmul(psF[:], ohpl[:, t, :], tgs[:, t, :],
                                                 start=(t == 0), stop=(t == tb - 1))
                            featst = bsc.tile([128, 129], F32, tag="featst")
                            nc.scalar.activation(out=featst[:], in_=psF[:], func=AF.Copy)
                            esc = bsc.tile([128, 1], F32, tag="esc")
                            nc.vector.tensor_scalar(out=esc[:], in0=featst[:, 128:129],
                                                    scalar1=1e-30, scalar2=None, op0=OP.max)
                            nc.vector.reciprocal(out=esc[:], in_=esc[:])
                            fs = bsc.tile([128, 128], F16, tag="fs")
                            nc.vector.tensor_tensor(
                                out=fs[:], in0=featst[:, 0:128],
                                in1=esc[:, 0:1].to_broadcast([128, 128]), op=OP.mult)
                            psT = bp2.tile([128, 128], F16, tag="psT")
                            nc.tensor.transpose(psT[:], fs[:], ident_sb[:])
                            fT = bsc.tile([128, 128], F16, tag="fT")
                            nc.vector.tensor_copy(out=fT[:], in_=psT[:])
                            psO = bp2.tile([128, 128], F32, tag="psO")
                            nc.tensor.matmul(psO[:], fT[:], fcw_sb[:], start=True, stop=True)
                            nc.vector.tensor_add(out=obA[:, s, :], in0=psO[:], in1=fcb_sb[:])

                        prev = None
                        for lo, hi in CH:
                            phase_a_group(lo, hi)
                            if prev is not None:
                                for s in range(*prev):
                                    phase_b(s)
                            prev = (lo, hi)
                        for s in range(*prev):
                            phase_b(s)

                    # ---------- batched L2 normalize + output ----------
                    with tc.tile_pool(name="fin", bufs=1) as fin:
                        scr = fin.tile([128, NSLOT, 128], F32)
                        nc.vector.tensor_mul(out=scr[:], in0=obA[:], in1=obA[:])
                        nrm = fin.tile([128, NSLOT], F32)
                        nc.vector.tensor_reduce(out=nrm[:], in_=scr[:],
                                                axis=mybir.AxisListType.X, op=OP.add)
                        nc.scalar.activation(out=nrm[:], in_=nrm[:], func=AF.Sqrt)
                        nc.vector.tensor_scalar(out=nrm[:], in0=nrm[:], scalar1=1e-12,
                                                scalar2=None, op0=OP.max)
                        nc.vector.reciprocal(out=nrm[:], in_=nrm[:])
                        obn = fin.tile([128, NSLOT, 128], F32)
                        nc.vector.tensor_tensor(
                            out=obn[:], in0=obA[:],
                            in1=nrm[:, :, None].to_broadcast([128, NSLOT, 128]),
                            op=OP.mult)
                        nc.sync.dma_start(
                            out=dt_out.ap().rearrange("(b p) d -> p b d", p=128),
                            in_=obn[:])

            if loop_k == 1:
                loop_body()
            else:
                with tc.For_i(0, loop_k, 1):
                    loop_body()

    nc.compile()
    if for_hw:
        nc.m = get_hw_module(nc.m)
    return nc


# ------------------------------------------------------------------- runner
class Runner:
    def __init__(self, nc, n_cores=NCORES):
        import jax
        from concourse.bass2jax import (_bass_exec_p, partition_id_tensor,
                                        install_neuronx_cc_hook)
        from jax.sharding import Mesh, PartitionSpec, NamedSharding
        from jax.experimental.shard_map import shard_map
        install_neuronx_cc_hook()
        self.jax = jax
        self.n_cores = n_cores
        pname = nc.partition_id_tensor.name if nc.partition_id_tensor else None
        in_names, out_names, out_avals = [], [], []
        for alloc in nc.m.functions[0].allocations:
            if not isinstance(alloc, mybir.MemoryLocationSet):
                continue
            name = alloc.memorylocations[0].name
            if alloc.kind == "ExternalInput":
                if name != pname:
                    in_names.append(name)
            elif alloc.kind == "ExternalOutput":
                out_names.append(name)
                out_avals.append(jax.core.ShapedArray(
                    tuple(alloc.tensor_shape), mybir.dt.np(alloc.dtype)))
        self.in_names, self.out_names, self.out_avals = in_names, out_names, out_avals
        n_params = len(in_names)
        all_in = list(in_names) + list(out_names)
        if pname is not None:
            all_in.append(pname)

        def _body(*args):
            operands = list(args)
            if pname is not None:
                operands.append(partition_id_tensor())
            return tuple(_bass_exec_p.bind(
                *operands, out_avals=tuple(out_avals), in_names=tuple(all_in),
                out_names=tuple(out_names), lowering_input_output_aliases=(),
                sim_require_finite=True, sim_require_nnan=True, nc=nc))

        devices = jax.devices()[:n_cores]
        self.mesh = Mesh(np.asarray(devices), ("core",))
        self.sharding = NamedSharding(self.mesh, PartitionSpec("core"))
        donate = tuple(range(n_params, n_params + len(out_names)))
        self.fn = jax.jit(shard_map(
            _body, mesh=self.mesh,
            in_specs=(PartitionSpec("core"),) * (n_params + len(out_names)),
            out_specs=(PartitionSpec("core"),) * len(out_names),
            check_rep=False), donate_argnums=donate, keep_unused=True)

    def put_inputs(self, in_maps):
        return [self.jax.device_put(
            np.concatenate([np.asarray(in_maps[c][nm]) for c in range(self.n_cores)], axis=0),
            self.sharding) for nm in self.in_names]

    def put_zeros(self):
        return [self.jax.device_put(
            np.zeros((self.n_cores * a.shape[0], *a.shape[1:]), a.dtype), self.sharding)
            for a in self.out_avals]

    def run(self, dev_in, dev_zeros):
        outs = self.fn(*dev_in, *dev_zeros)
        self.jax.block_until_ready(outs)
        return outs

    def unpack(self, outs):
        return [{nm: np.asarray(outs[i]).reshape(self.n_cores, *self.out_avals[i].shape)[c]
                 for i, nm in enumerate(self.out_names)} for c in range(self.n_cores)]


_CACHE = {}


def _get_runner(spec, loop_k=1):
    key = (spec, loop_k)
    if key not in _CACHE:
        nc = build_program(spec, loop_k)
        _CACHE[key] = Runner(nc)
    return _CACHE[key]


def kernel(**inputs):
    in_maps, meta = _host_prep(**inputs)
    r = _get_runner(meta["t_b"], 1)
    dev = r.put_inputs(in_maps)
    res = r.unpack(r.run(dev, r.put_zeros()))
    assign = meta["assign"]
    out = np.empty((N, D), np.float32)
    for k in range(NCORES):
        ok = res[k]["out"]
        for s in range(NSLOT):
            bi = assign[k, s]
            if bi < 0:
                continue
            b0 = 128 * bi
            nn = min(128, N - b0)
            out[b0:b0 + nn] = ok[128 * s:128 * s + nn]
    return out


# revision 25
# speedup vs baseline: 1.0326x; 1.0326x over previous
"""GAT layer kernel for Trainium2, 8 NeuronCores — v7.

v6 -> v7 (LDW queue ~97% saturated, but ~90us of per-slot phase-B gaps
  where PE waits on the DVE is_eq+tgs chain):
  - scatter one-hot `ohpl` now comes from the HOST (fp8 DMA, +10.7MB/core)
    instead of a 4.7us/slot DVE is_equal;
  - phase B is interleaved per chunk-group (A-group -> finish -> B-slots)
    so B's DVE/PE work overlaps the next group's LDW stream.
"""

_OLD_DOC = """GAT layer kernel for Trainium2, 8 NeuronCores — v6.

v3 -> v4: killed the device dma_gather (21 x ~34us GpSimd descriptor-gen
  serial stream) by host-gathering h[src] into per-edge streams; per-edge
  eh via LN-stats on the gathered rows; et shrunk to own dst nodes.
  1122us -> 495us.

v4 -> v5: fp8 stationaries — MEASURED NO EFFECT: LDWEIGHTS is ~104ns per
  128x128 stationary regardless of dtype (row-streaming at ~1.2GHz).
  GpSimd tgs-multiply regressed (519us).  Kept: fp8 squares/one-hots
  (halve their DMA), both squares on ACT.

v5 -> v6 (LDWEIGHTS-count is the wall: 6 streams x chunk-count x 104ns):
  - Rebalanced dst-block assignment: 157 blocks packed into 8 cores x 20
    slots (was 21) with per-slot chunk counts TB[s] = ceil(max block size
    in slot / 128); flat-packed streams.  Sum(TB) 651 vs 714 (-8.8%).
  - hg_pk rows are 129-wide with a host 1.0 column: one broadcast multiply
    writes all 129 tgs columns (kills a 1.9us strided CAST per block).
  - fs scale via broadcast tensor_tensor (AP-scalar tensor_scalar was
    ~1.1us); LN constants c_h/c_t/c_r as float immediates.
  - Stat-major flat arrays (suR/s1R/suH/s1H/etd/s2R/s2H) so batched
    finishes read contiguous ranges; ACT calls grouped by function.
"""

import os
import sys

sys.path.insert(0, "/opt/trn_rl_repo")

import numpy as np
import ml_dtypes

import concourse.bacc as bacc
import concourse.bass as bass
import concourse.mybir as mybir
import concourse.tile as tile
from concourse.bass_interp import get_hw_module

F32 = mybir.dt.float32
F16 = mybir.dt.float16
F8 = mybir.dt.float8e4
AF = mybir.ActivationFunctionType
OP = mybir.AluOpType
E4 = ml_dtypes.float8_e4m3

N = 20000
E = 640000
D = 128
NCORES = 8
EPS = 1e-6
NSLOT = 20


# ----------------------------------------------------------------- host prep
def _host_prep(h, r, src, dst, hn_a, hn_b, tn_a, tn_b, rn_a, rn_b,
               head_w, tail_w, rel_w, fc_w, fc_b):
    h = np.asarray(h, np.float32); r = np.asarray(r, np.float32)
    src = np.asarray(src, np.int32); dst = np.asarray(dst, np.int32)

    u_h = np.asarray(hn_a, np.float32) * np.asarray(head_w, np.float32)
    u_t = np.asarray(tn_a, np.float32) * np.asarray(tail_w, np.float32)
    u_r = np.asarray(rn_a, np.float32) * np.asarray(rel_w, np.float32)
    w_h = u_h - u_h.sum() / D
    w_t = u_t - u_t.sum() / D
    w_r = u_r - u_r.sum() / D
    c_h = float((np.asarray(hn_b, np.float32) * head_w).sum())
    c_t = float((np.asarray(tn_b, np.float32) * tail_w).sum())
    c_r = float((np.asarray(rn_b, np.float32) * rel_w).sum())

    perm = np.argsort(dst, kind="stable")
    dst_s = dst[perm]; src_s = src[perm]
    counts = np.bincount(dst, minlength=N)
    cum = np.concatenate([[0], np.cumsum(counts)])

    # --- balanced block -> (core, slot) assignment -----------------------
    nblk = (N + 127) // 128                     # 157
    bcnt = np.array([int(cum[min(b0 + 128, N)] - cum[b0])
                     for b0 in range(0, N, 128)])
    order = np.argsort(-bcnt, kind="stable")    # blocks desc by edge count
    # slot s gets ranks [8s, 8s+8); within a slot, largest block goes to the
    # currently least-loaded core
    assign = -np.ones((NCORES, NSLOT), np.int64)
    load = np.zeros(NCORES, np.int64)
    TB = []
    for s in range(NSLOT):
        grp = list(order[8 * s: 8 * s + 8])
        mx = max((bcnt[bi] for bi in grp), default=0)
        TB.append(max(1, (int(mx) + 127) // 128))
        cores = np.argsort(load, kind="stable")
        for i, bi in enumerate(grp):
            assign[cores[i], s] = bi
            load[cores[i]] += bcnt[bi]
    TB = tuple(TB)
    toff = np.concatenate([[0], np.cumsum(TB)])
    EPKf = int(toff[-1])                        # total chunk slots per core
    ef = 128 * EPKf                             # total edge slots per core

    # zero-padded f16 copies for host-side gathers
    h16z = np.zeros((N + 1, D + 1), np.float16)
    h16z[:N, :D] = h.astype(np.float16)
    h16z[:N, D] = 1.0                           # tgs esum column
    h16zT = np.ascontiguousarray(h16z[:, :D].T)     # [128, N+1]
    r16z = np.zeros((E + 1, D), np.float16)
    r16z[:E] = r.astype(np.float16)

    iota16 = np.broadcast_to(np.arange(128, dtype=np.float16), (128, 128)).copy()
    ident = np.eye(128, dtype=np.float16)

    def wcol(w):
        a = np.zeros((128, 2), np.float16)
        a[:, 0] = w.astype(np.float16); a[:, 1] = 1.0
        return a
    wr16 = wcol(w_r); wh16 = wcol(w_h); wt16 = wcol(w_t)
    ones16 = np.ones((128, 1), np.float16)
    ones2 = np.zeros((128, 2, 2), np.float32)
    ones2[:, 0, 0] = 1.0; ones2[:, 1, 1] = 1.0
    ones2 = ones2.reshape(128, 4).astype(E4)
    fcw16 = np.asarray(fc_w, np.float32).astype(np.float16)
    fcb = np.broadcast_to(np.asarray(fc_b, np.float32), (128, 128)).copy()

    rep = {"iota16": iota16, "ident": ident, "wr16": wr16, "wh16": wh16,
           "wt16": wt16, "ones16": ones16, "ones2": ones2, "fcw16": fcw16,
           "fcb": fcb}

    vidx = np.arange(128, dtype=np.float32)

    in_maps = []
    for k in range(NCORES):
        src_arr = np.full((EPKf, 128), N, np.int64)     # pad -> zero row
        rcol = np.full((EPKf, 128), E, np.int64)
        dstl = np.full((EPKf, 128), -1.0, np.float32)   # [chunk, p]
        hTo = np.zeros((D, NSLOT * 128), np.float16)
        for s in range(NSLOT):
            bi = assign[k, s]
            if bi < 0:
                continue
            b0 = 128 * bi
            e0, e1 = int(cum[b0]), int(cum[min(b0 + 128, N)])
            cnt = e1 - e0
            o = toff[s]
            # edge i (0..cnt) at chunk o + i//128, lane i%128
            fl = np.full(TB[s] * 128, N, np.int64)
            fl[:cnt] = src_s[e0:e1]
            src_arr[o:o + TB[s]] = fl.reshape(TB[s], 128)
            fl = np.full(TB[s] * 128, E, np.int64)
            fl[:cnt] = perm[e0:e1]
            rcol[o:o + TB[s]] = fl.reshape(TB[s], 128)
            fl = np.full(TB[s] * 128, -1.0, np.float32)
            fl[:cnt] = (dst_s[e0:e1] - b0).astype(np.float32)
            dstl[o:o + TB[s]] = fl.reshape(TB[s], 128)
            nn = min(128, N - b0)
            hTo[:, 128 * s:128 * s + nn] = h16zT[:, b0:b0 + nn]

        rT16 = np.ascontiguousarray(r16z[rcol.reshape(-1)].T)
        hgT16 = np.ascontiguousarray(h16zT[:, src_arr.reshape(-1)])
        # hg_pk[p, c, :] = h16z[src of edge (chunk c, lane p)] with ones col
        hg = h16z[src_arr]                               # [c, p, 129]
        hg_pk = np.ascontiguousarray(
            hg.transpose(1, 0, 2).reshape(128, EPKf * (D + 1)))
        dstl_pk = np.ascontiguousarray(dstl.T)           # [128, EPKf]
        oht = (dstl.reshape(1, -1) == vidx[:, None])     # [j, c*128]
        oht = np.ascontiguousarray(oht).astype(E4)
        # scatter one-hot, pk layout: ohpl[p, c, j] = (dstl[c, p] == j)
        ohpl = (dstl[:, :, None] == vidx[None, None, :])  # [c, p, j]
        ohpl = np.ascontiguousarray(
            ohpl.transpose(1, 0, 2).reshape(128, EPKf * 128)).astype(E4)

        in_maps.append(dict(rep, rT16=rT16, hgT16=hgT16, hg_pk=hg_pk,
                            dstl=dstl_pk, oht=oht, ohpl=ohpl, hTo=hTo))
    spec = (TB, round(c_h, 8), round(c_t, 8), round(c_r, 8))
    meta = dict(t_b=spec, assign=assign, ef=ef)
    return in_maps, meta


# ------------------------------------------------------------ device program
def build_program(spec, loop_k=1, for_hw=True):
    TB, c_h, c_t, c_r = spec
    TBm = max(TB)
    toff = [0]
    for t in TB:
        toff.append(toff[-1] + t)
    EPKf = toff[-1]
    ef = 128 * EPKf
    nc = bacc.Bacc("TRN2", target_bir_lowering=False, debug=False,
                   enable_asserts=False, num_devices=NCORES if for_hw else 1)

    dt_rT = nc.dram_tensor("rT16", [128, ef], F16, kind="ExternalInput")
    dt_hgT = nc.dram_tensor("hgT16", [128, ef], F16, kind="ExternalInput")
    dt_hg = nc.dram_tensor("hg_pk", [128, EPKf * (D + 1)], F16, kind="ExternalInput")
    dt_oht = nc.dram_tensor("oht", [128, ef], F8, kind="ExternalInput")
    dt_ohpl = nc.dram_tensor("ohpl", [128, EPKf * 128], F8, kind="ExternalInput")
    dt_hTo = nc.dram_tensor("hTo", [128, NSLOT * 128], F16, kind="ExternalInput")
    dt_ident = nc.dram_tensor("ident", [128, 128], F16, kind="ExternalInput")
    dt_wr = nc.dram_tensor("wr16", [128, 2], F16, kind="ExternalInput")
    dt_wh = nc.dram_tensor("wh16", [128, 2], F16, kind="ExternalInput")
    dt_wt = nc.dram_tensor("wt16", [128, 2], F16, kind="ExternalInput")
    dt_ones = nc.dram_tensor("ones16", [128, 1], F16, kind="ExternalInput")
    dt_ones2 = nc.dram_tensor("ones2", [128, 4], F8, kind="ExternalInput")
    dt_fcw = nc.dram_tensor("fcw16", [128, 128], F16, kind="ExternalInput")
    dt_fcb = nc.dram_tensor("fcb", [128, 128], F32, kind="ExternalInput")
    dt_out = nc.dram_tensor("out", [NSLOT * 128, 128], F32, kind="ExternalOutput")

    with tile.TileContext(nc) as tc:
        with tc.tile_pool(name="const", bufs=1) as cpool:
            ident_sb = cpool.tile([128, 128], F16)
            nc.sync.dma_start(out=ident_sb[:], in_=dt_ident.ap())
            wr_sb = cpool.tile([128, 2], F16)
            nc.sync.dma_start(out=wr_sb[:], in_=dt_wr.ap())
            wh_sb = cpool.tile([128, 2], F16)
            nc.sync.dma_start(out=wh_sb[:], in_=dt_wh.ap())
            wt_sb = cpool.tile([128, 2], F16)
            nc.sync.dma_start(out=wt_sb[:], in_=dt_wt.ap())
            ones_sb = cpool.tile([128, 1], F16)
            nc.sync.dma_start(out=ones_sb[:], in_=dt_ones.ap())
            ones2_sb = cpool.tile([128, 2, 2], F8)
            nc.sync.dma_start(out=ones2_sb[:],
                              in_=dt_ones2.ap().rearrange("p (a b) -> p a b", a=2))
            fcw_sb = cpool.tile([128, 128], F16)
            nc.sync.dma_start(out=fcw_sb[:], in_=dt_fcw.ap())
            fcb_sb = cpool.tile([128, 128], F32)
            nc.sync.dma_start(out=fcb_sb[:], in_=dt_fcb.ap())

            def loop_body():
                with tc.tile_pool(name="stage", bufs=1) as st:
                    et16 = st.tile([128, NSLOT], F16)
                    suR = st.tile([128, EPKf], F32)
                    s1R = st.tile([128, EPKf], F32)
                    suH = st.tile([128, EPKf], F32)
                    s1H = st.tile([128, EPKf], F32)
                    etdA = st.tile([128, EPKf], F32)
                    s2R = st.tile([128, EPKf], F32)
                    s2H = st.tile([128, EPKf], F32)
                    exA = st.tile([128, EPKf], F32)
                    obA = st.tile([128, NSLOT, 128], F16)

                    # ---------- node et stats (own dst blocks only) ----------
                    with tc.tile_pool(name="nwork", bufs=1) as nw, \
                         tc.tile_pool(name="npsum", bufs=1, space="PSUM") as npp:
                        hTo_sb = nw.tile([128, NSLOT * 128], F16)
                        nc.sync.dma_start(out=hTo_sb[:], in_=dt_hTo.ap())
                        hTo2 = nw.tile([128, NSLOT * 128], F8)
                        nc.vector.tensor_mul(out=hTo2[:], in0=hTo_sb[:], in1=hTo_sb[:])
                        psN = npp.tile([128, 3 * NSLOT], F32)
                        for c in range(NSLOT):
                            nc.tensor.matmul(psN[:, 2 * c:2 * c + 2],
                                             hTo_sb[:, 128 * c:128 * (c + 1)],
                                             wt_sb[:], start=True, stop=True)
                            nc.tensor.matmul(psN[:, 2 * NSLOT + c:2 * NSLOT + c + 1],
                                             hTo2[:, 128 * c:128 * (c + 1)],
                                             ones_sb[:], start=True, stop=True)
                        sN = nw.tile([128, 3 * NSLOT], F32)
                        nc.vector.tensor_copy(out=sN[:], in_=psN[:])
                        suN = sN[:, 0:2 * NSLOT:2]
                        s1N = sN[:, 1:2 * NSLOT:2]
                        s2N = sN[:, 2 * NSLOT:3 * NSLOT]
                        t0 = nw.tile([128, NSLOT], F32)
                        nc.vector.tensor_mul(out=t0[:], in0=s1N, in1=s1N)
                        nc.vector.tensor_scalar_mul(out=t0[:], in0=t0[:], scalar1=-1.0 / 128.0)
                        nc.vector.tensor_tensor(out=t0[:], in0=t0[:], in1=s2N, op=OP.add)
                        nc.scalar.activation(out=t0[:], in_=t0[:], func=AF.Sqrt, scale=1.0 / 127.0)
                        nc.vector.tensor_scalar_add(out=t0[:], in0=t0[:], scalar1=EPS)
                        nc.vector.reciprocal(out=t0[:], in_=t0[:])
                        m1 = nw.tile([128, NSLOT], F32)
                        nc.vector.tensor_tensor(out=m1[:], in0=suN, in1=t0[:], op=OP.mult)
                        nc.vector.tensor_scalar_add(out=m1[:], in0=m1[:], scalar1=c_t)
                        nc.scalar.activation(out=et16[:], in_=m1[:], func=AF.Tanh)

                    # ---- phase A: per-edge stats + batched finish -> exA ----
                    CH = [(0, 6), (6, 12), (12, 17), (17, NSLOT)]
                    with tc.tile_pool(name="awork", bufs=2) as aw, \
                         tc.tile_pool(name="bwork", bufs=2) as bw, \
                         tc.tile_pool(name="hgp", bufs=3) as hgp, \
                         tc.tile_pool(name="ohplp", bufs=3) as ohp, \
                         tc.tile_pool(name="bsc", bufs=3) as bsc, \
                         tc.tile_pool(name="efin", bufs=2) as ef_, \
                         tc.tile_pool(name="apsum", bufs=2, space="PSUM") as ap_, \
                         tc.tile_pool(name="bps1", bufs=2, space="PSUM") as bp1, \
                         tc.tile_pool(name="bps2", bufs=2, space="PSUM") as bp2:
                        def phase_a_group(lo, hi):
                            for s in range(lo, hi):
                                tb = TB[s]
                                off = toff[s]
                                rTb = aw.tile([128, TBm * 128], F16, tag="rTb")
                                nc.scalar.dma_start(out=rTb[:, :128 * tb],
                                                    in_=dt_rT.ap()[:, 128 * off:128 * (off + tb)])
                                hgTb = aw.tile([128, TBm * 128], F16, tag="hgTb")
                                nc.sync.dma_start(out=hgTb[:, :128 * tb],
                                                  in_=dt_hgT.ap()[:, 128 * off:128 * (off + tb)])
                                ohtb = aw.tile([128, TBm * 128], F8, tag="ohtb")
                                nc.scalar.dma_start(out=ohtb[:, :128 * tb],
                                                    in_=dt_oht.ap()[:, 128 * off:128 * (off + tb)])
                                psE = ap_.tile([128, 7 * TBm], F32, tag="psE")
                                for t in range(tb):
                                    nc.tensor.matmul(psE[:, 2 * t:2 * t + 2],
                                                     rTb[:, 128 * t:128 * (t + 1)],
                                                     wr_sb[:], start=True, stop=True)
                                    nc.tensor.matmul(psE[:, 2 * tb + 2 * t:2 * tb + 2 * t + 2],
                                                     hgTb[:, 128 * t:128 * (t + 1)],
                                                     wh_sb[:], start=True, stop=True)
                                sq2 = aw.tile([128, TBm, 2, 128], F8, tag="sq2")
                                nc.scalar.activation(
                                    out=sq2[:, :tb, 0, :],
                                    in_=rTb[:, :128 * tb].rearrange("p (t d) -> p t d", t=tb),
                                    func=AF.Square)
                                nc.scalar.activation(
                                    out=sq2[:, :tb, 1, :],
                                    in_=hgTb[:, :128 * tb].rearrange("p (t d) -> p t d", t=tb),
                                    func=AF.Square)
                                for t in range(tb):
                                    nc.tensor.matmul(psE[:, 5 * tb + 2 * t:5 * tb + 2 * t + 2],
                                                     sq2[:, t, :, :], ones2_sb[:],
                                                     perf_mode=mybir.MatmulPerfMode.DoubleRow,
                                                     start=True, stop=True)
                                for t in range(tb):
                                    nc.tensor.matmul(psE[:, 4 * tb + t:4 * tb + t + 1],
                                                     ohtb[:, 128 * t:128 * (t + 1)],
                                                     et16[:, s:s + 1], start=True, stop=True)
                                for arr, sl in ((suR, slice(0, 2 * tb, 2)),
                                                (s1R, slice(1, 2 * tb, 2)),
                                                (suH, slice(2 * tb, 4 * tb, 2)),
                                                (s1H, slice(2 * tb + 1, 4 * tb, 2)),
                                                (etdA, slice(4 * tb, 5 * tb)),
                                                (s2R, slice(5 * tb, 7 * tb, 2)),
                                                (s2H, slice(5 * tb + 1, 7 * tb, 2))):
                                    nc.vector.tensor_copy(out=arr[:, off:off + tb], in_=psE[:, sl])
                            # batched finish for slots [lo, hi)
                            o0, o1 = toff[lo], toff[hi]
                            cw = o1 - o0
                            tR = ef_.tile([128, 7 * TBm], F32, tag="tR")
                            tRv = tR[:, 0:cw]
                            tH = ef_.tile([128, 7 * TBm], F32, tag="tH")
                            tHv = tH[:, 0:cw]
                            nc.vector.tensor_tensor(out=tRv, in0=s1R[:, o0:o1], in1=s1R[:, o0:o1], op=OP.mult)
                            nc.vector.tensor_scalar_mul(out=tRv, in0=tRv, scalar1=-1.0 / 128.0)
                            nc.vector.tensor_tensor(out=tRv, in0=tRv, in1=s2R[:, o0:o1], op=OP.add)
                            nc.vector.tensor_tensor(out=tHv, in0=s1H[:, o0:o1], in1=s1H[:, o0:o1], op=OP.mult)
                            nc.vector.tensor_scalar_mul(out=tHv, in0=tHv, scalar1=-1.0 / 128.0)
                            nc.vector.tensor_tensor(out=tHv, in0=tHv, in1=s2H[:, o0:o1], op=OP.add)
                            nc.scalar.activation(out=tRv, in_=tRv, func=AF.Sqrt, scale=1.0 / 127.0)
                            nc.scalar.activation(out=tHv, in_=tHv, func=AF.Sqrt, scale=1.0 / 127.0)
                            nc.vector.tensor_scalar_add(out=tRv, in0=tRv, scalar1=EPS)
                            nc.vector.tensor_scalar_add(out=tHv, in0=tHv, scalar1=EPS)
                            nc.vector.reciprocal(out=tRv, in_=tRv)
                            nc.vector.reciprocal(out=tHv, in_=tHv)
                            nc.vector.tensor_tensor(out=tRv, in0=suR[:, o0:o1], in1=tRv, op=OP.mult)
                            nc.vector.tensor_tensor(out=tHv, in0=suH[:, o0:o1], in1=tHv, op=OP.mult)
                            nc.vector.tensor_scalar_add(out=tRv, in0=tRv, scalar1=c_r)
                            nc.vector.tensor_scalar_add(out=tHv, in0=tHv, scalar1=c_h)
                            nc.scalar.activation(out=tRv, in_=tRv, func=AF.Tanh)
                            nc.scalar.activation(out=tHv, in_=tHv, func=AF.Tanh)
                            nc.vector.tensor_tensor(out=tRv, in0=tRv, in1=tHv, op=OP.add)
                            exv = exA[:, o0:o1]
                            nc.vector.tensor_tensor(out=exv, in0=tRv, in1=etdA[:, o0:o1], op=OP.add)
                            nc.scalar.activation(out=exv, in_=exv, func=AF.Exp)
                            nc.vector.tensor_scalar(out=exv, in0=exv, scalar1=1.0,
                                                    scalar2=None, op0=OP.max)

                        def phase_b(s):
                            tb = TB[s]
                            off = toff[s]
                            ohpl = ohp.tile([128, TBm, 128], F8, tag="ohpl")
                            nc.sync.dma_start(
                                out=ohpl[:, :tb, :],
                                in_=dt_ohpl.ap()[:, off * 128:(off + tb) * 128]
                                    .rearrange("p (t d) -> p t d", t=tb))
                            hgb = hgp.tile([128, TBm, D + 1], F16, tag="hgb")
                            nc.sync.dma_start(
                                out=hgb[:, :tb, :],
                                in_=dt_hg.ap()[:, off * (D + 1):(off + tb) * (D + 1)]
                                    .rearrange("p (t d) -> p t d", t=tb))
                            tgs = bw.tile([128, TBm, D + 1], F16, tag="tgs")
                            nc.vector.tensor_tensor(
                                out=tgs[:, :tb, :], in0=hgb[:, :tb, :],
                                in1=exA[:, off:off + tb, None].to_broadcast([128, tb, D + 1]),
                                op=OP.mult)
                            psF = bp1.tile([128, 129], F32, tag="psF")
                            for t in range(tb):
                                nc.tensor.matmul(psF[:], ohpl[:, t, :], tgs[:, t, :],
                                                 start=(t == 0), stop=(t == tb - 1))
                            featst = bsc.tile([128, 129], F32, tag="featst")
                            nc.scalar.activation(out=featst[:], in_=psF[:], func=AF.Copy)
                            esc = bsc.tile([128, 1], F32, tag="esc")
                            nc.vector.tensor_scalar(out=esc[:], in0=featst[:, 128:129],
                                                    scalar1=1e-30, scalar2=None, op0=OP.max)
                            nc.vector.reciprocal(out=esc[:], in_=esc[:])
                            fs = bsc.tile([128, 128], F16, tag="fs")
                            nc.vector.tensor_tensor(
                                out=fs[:], in0=featst[:, 0:128],
                                in1=esc[:, 0:1].to_broadcast([128, 128]), op=OP.mult)
                            psT = bp2.tile([128, 128], F16, tag="psT")
                            nc.tensor.transpose(psT[:], fs[:], ident_sb[:])
                            fT = bsc.tile([128, 128], F16, tag="fT")
                            nc.vector.tensor_copy(out=fT[:], in_=psT[:])
                            psO = bp2.tile([128, 128], F32, tag="psO")
                            nc.tensor.matmul(psO[:], fT[:], fcw_sb[:], start=True, stop=True)
                            nc.vector.tensor_add(out=obA[:, s, :], in0=psO[:], in1=fcb_sb[:])

                        prev = None
                        for lo, hi in CH:
                            phase_a_group(lo, hi)
                            if prev is not None:
                                for s in range(*prev):
                                    phase_b(s)
                            prev = (lo, hi)
                        for s in range(*prev):
                            phase_b(s)

                    # ---------- batched L2 normalize + output ----------
                    with tc.tile_pool(name="fin", bufs=1) as fin:
                        scr = fin.tile([128, NSLOT, 128], F32)
                        nc.vector.tensor_mul(out=scr[:], in0=obA[:], in1=obA[:])
                        nrm = fin.tile([128, NSLOT], F32)
                        nc.vector.tensor_reduce(out=nrm[:], in_=scr[:],
                                                axis=mybir.AxisListType.X, op=OP.add)
                        nc.scalar.activation(out=nrm[:], in_=nrm[:], func=AF.Sqrt)
                        nc.vector.tensor_scalar(out=nrm[:], in0=nrm[:], scalar1=1e-12,
                                                scalar2=None, op0=OP.max)
                        nc.vector.reciprocal(out=nrm[:], in_=nrm[:])
                        obn = fin.tile([128, NSLOT, 128], F32)
                        nc.vector.tensor_tensor(
                            out=obn[:], in0=obA[:],
                            in1=nrm[:, :, None].to_broadcast([128, NSLOT, 128]),
                            op=OP.mult)
                        nc.sync.dma_start(
                            out=dt_out.ap().rearrange("(b p) d -> p b d", p=128),
                            in_=obn[:])

            if loop_k == 1:
                loop_body()
            else:
                with tc.For_i(0, loop_k, 1):
                    loop_body()

    nc.compile()
    if for_hw:
        nc.m = get_hw_module(nc.m)
    return nc


# ------------------------------------------------------------------- runner
class Runner:
    def __init__(self, nc, n_cores=NCORES):
        import jax
        from concourse.bass2jax import (_bass_exec_p, partition_id_tensor,
                                        install_neuronx_cc_hook)
        from jax.sharding import Mesh, PartitionSpec, NamedSharding
        from jax.experimental.shard_map import shard_map
        install_neuronx_cc_hook()
        self.jax = jax
        self.n_cores = n_cores
        pname = nc.partition_id_tensor.name if nc.partition_id_tensor else None
        in_names, out_names, out_avals = [], [], []
        for alloc in nc.m.functions[0].allocations:
            if not isinstance(alloc, mybir.MemoryLocationSet):
                continue
            name = alloc.memorylocations[0].name
            if alloc.kind == "ExternalInput":
                if name != pname:
                    in_names.append(name)
            elif alloc.kind == "ExternalOutput":
                out_names.append(name)
                out_avals.append(jax.core.ShapedArray(
                    tuple(alloc.tensor_shape), mybir.dt.np(alloc.dtype)))
        self.in_names, self.out_names, self.out_avals = in_names, out_names, out_avals
        n_params = len(in_names)
        all_in = list(in_names) + list(out_names)
        if pname is not None:
            all_in.append(pname)

        def _body(*args):
            operands = list(args)
            if pname is not None:
                operands.append(partition_id_tensor())
            return tuple(_bass_exec_p.bind(
                *operands, out_avals=tuple(out_avals), in_names=tuple(all_in),
                out_names=tuple(out_names), lowering_input_output_aliases=(),
                sim_require_finite=True, sim_require_nnan=True, nc=nc))

        devices = jax.devices()[:n_cores]
        self.mesh = Mesh(np.asarray(devices), ("core",))
        self.sharding = NamedSharding(self.mesh, PartitionSpec("core"))
        donate = tuple(range(n_params, n_params + len(out_names)))
        self.fn = jax.jit(shard_map(
            _body, mesh=self.mesh,
            in_specs=(PartitionSpec("core"),) * (n_params + len(out_names)),
            out_specs=(PartitionSpec("core"),) * len(out_names),
            check_rep=False), donate_argnums=donate, keep_unused=True)

    def put_inputs(self, in_maps):
        return [self.jax.device_put(
            np.concatenate([np.asarray(in_maps[c][nm]) for c in range(self.n_cores)], axis=0),
            self.sharding) for nm in self.in_names]

    def put_zeros(self):
        return [self.jax.device_put(
            np.zeros((self.n_cores * a.shape[0], *a.shape[1:]), a.dtype), self.sharding)
            for a in self.out_avals]

    def run(self, dev_in, dev_zeros):
        outs = self.fn(*dev_in, *dev_zeros)
        self.jax.block_until_ready(outs)
        return outs

    def unpack(self, outs):
        return [{nm: np.asarray(outs[i]).reshape(self.n_cores, *self.out_avals[i].shape)[c]
                 for i, nm in enumerate(self.out_names)} for c in range(self.n_cores)]


_CACHE = {}


def _get_runner(spec, loop_k=1):
    key = (spec, loop_k)
    if key not in _CACHE:
        nc = build_program(spec, loop_k)
        _CACHE[key] = Runner(nc)
    return _CACHE[key]


def kernel(**inputs):
    in_maps, meta = _host_prep(**inputs)
    r = _get_runner(meta["t_b"], 1)
    dev = r.put_inputs(in_maps)
    res = r.unpack(r.run(dev, r.put_zeros()))
    assign = meta["assign"]
    out = np.empty((N, D), np.float32)
    for k in range(NCORES):
        ok = res[k]["out"]
        for s in range(NSLOT):
            bi = assign[k, s]
            if bi < 0:
                continue
            b0 = 128 * bi
            nn = min(128, N - b0)
            out[b0:b0 + nn] = ok[128 * s:128 * s + nn]
    return out
